# revision 13
# baseline (speedup 1.0000x reference)
"""Trainium2 Bass kernel for nn_Attention_49606872268904.

Dense causal GQA attention block (B=1, S=2048, D=4096, 32 q-heads, 8 kv-heads,
head_dim=128, rope, causal mask, output projection), tensor-parallel over heads
across 8 NeuronCores: core c owns q-heads 4c..4c+3 and kv-head c. Each core
computes its partial output projection; a chunked ReduceScatter sums partials
and leaves each core with 1/8 of the output rows (d-dim), assembled on host.

Layout notes:
- All matmuls run in "transposed" orientation: activations live as [feature, seq]
  so the contraction dim is always on SBUF partitions.
- RoPE uses the permuted-weight trick: wq/wk rows are reordered so each head's
  output dims are [real_0..real_63, imag_0..imag_63]; rotation is then two
  contiguous column blocks instead of a stride-2 interleave. Scores are
  invariant to the within-head permutation.
- Softmax is computed without max subtraction (scores are tiny for this
  problem's data scale; masked entries are exact zeros via a multiplicative
  triangular mask after exp).
- Causality: score blocks strictly above the diagonal are skipped entirely.
"""

import numpy as np

import concourse.bass as bass
import concourse.mybir as mybir
import concourse.tile as tile
from concourse import bacc
from concourse.bass_utils import run_bass_kernel_spmd
from concourse.masks import make_identity, make_upper_triangular

B, S, DIM = 1, 2048, 4096
NH, NKV, HD = 32, 8, 128
N_CORES = 8
HPC = NH // N_CORES          # 4 q heads per core
OPC = HPC * HD               # 512 output dims per core
DCH = DIM // 128             # 32 contraction chunks
SW = 512                     # seq group width
NSG = S // SW                # 4 seq groups
SCALE = float(HD) ** -0.5

DT = mybir.dt.float32
DTR = mybir.dt.float32r
FP = mybir.ActivationFunctionType

_cached = None
last_results = None  # BassKernelResults of the most recent run (for test harness)


def build_program():
    nc = bacc.Bacc(
        "TRN2",
        target_bir_lowering=False,
        debug=False,
        enable_asserts=False,
        num_devices=N_CORES,
    )

    xT = nc.declare_dram_parameter("xT", [DIM, S], DTR, isOutput=False)
    wqT = nc.declare_dram_parameter("wqT", [DIM, OPC], DTR, isOutput=False)
    wkT = nc.declare_dram_parameter("wkT", [DIM, HD], DTR, isOutput=False)
    wvT = nc.declare_dram_parameter("wvT", [DIM, HD], DTR, isOutput=False)
    woT = nc.declare_dram_parameter("woT", [32, 128, OPC], DTR, isOutput=False)
    cos2 = nc.declare_dram_parameter("cos2", [128, S], DT, isOutput=False)
    sinpm = nc.declare_dram_parameter("sinpm", [128, S], DT, isOutput=False)
    y_out = nc.declare_dram_parameter("y_shard", [4, 512, 512], DT, isOutput=True)

    xT_r = xT.rearrange("(g p) s -> p g s", p=128)     # [128, 32, S]
    wq_r = wqT.rearrange("(g p) o -> p g o", p=128)    # [128, 32, 512]
    wk_r = wkT.rearrange("(g p) o -> p g o", p=128)    # [128, 32, 128]
    wv_r = wvT.rearrange("(g p) o -> p g o", p=128)

    with tile.TileContext(nc) as tc:
        with (
            tc.tile_pool(name="dram", bufs=1, space="DRAM") as dram,
            tc.tile_pool(name="consts", bufs=1) as consts,
            tc.tile_pool(name="persist", bufs=1) as persist,
        ):
            yT_st = [
                dram.tile([DIM, SW], DT, name=f"ytс{r}") for r in range(4)
            ]
            rs_outs = [
                dram.tile([512, SW], DT, name=f"rso{r}") for r in range(4)
            ]

            ident = consts.tile([128, 128], DT)
            make_identity(nc, ident)
            tri_keep = consts.tile([128, 128], DT)
            make_upper_triangular(nc, tri_keep, val=1.0, diag=True)
            ones_f = consts.tile([128, 1], DT)
            nc.gpsimd.memset(ones_f, 1.0)
            ones_col = consts.tile([128, 1], DTR)
            nc.vector.tensor_copy(ones_col, ones_f)
            cos2_sb = consts.tile([128, S], DT)
            nc.sync.dma_start(cos2_sb, cos2[:])
            sinpm_sb = consts.tile([128, S], DT)
            nc.sync.dma_start(sinpm_sb, sinpm[:])

            KT_sb = persist.tile([128, S], DTR)       # K_rot^T, all kv positions
            V_sb = persist.tile([128, S], DTR)        # V blocks [kv, hd] at col 128j
            attnT = [persist.tile([128, S], DTR, name=f"attnT{h}") for h in range(HPC)]
            q_tiles = {}

            # ---------------- Phase P: QKV projections + RoPE ----------------
            with (
                nc.named_scope("phaseP"),
                tc.tile_pool(name="psP", bufs=1, space="PSUM") as psP,
                tc.tile_pool(name="sbP", bufs=1) as sbP,
                tc.tile_pool(name="qpool", bufs=1) as qpool,
            ):
                wq_sb = sbP.tile([128, DCH * OPC], DTR)  # resident wqT (64KB/part)
                wq_v = wq_sb.rearrange("p (g o) -> p g o", o=OPC)

                for sg in range(NSG):
                    scol = slice(sg * SW, (sg + 1) * SW)
                    q_ps = [
                        psP.tile([128, SW], DT, tag=f"q{h}", name=f"qps_{sg}_{h}")
                        for h in range(HPC)
                    ]
                    k_ps = psP.tile([128, SW], DT, tag="k", name=f"kps_{sg}")
                    v_ps = psP.tile([128, SW], DT, tag="v", name=f"vps_{sg}")
                    for dg in range(8):
                        if sg == 0:
                            nc.sync.dma_start(
                                wq_v[:, 4 * dg : 4 * dg + 4], wq_r[:, 4 * dg : 4 * dg + 4]
                            )
                        xg = sbP.tile([128, 2048], DTR, tag="xg", bufs=2, name=f"xg_{sg}_{dg}")
                        nc.sync.dma_start(
                            xg.rearrange("p (c s) -> p c s", s=SW),
                            xT_r[:, 4 * dg : 4 * dg + 4, scol],
                        )
                        kg = sbP.tile([128, 512], DTR, tag="kg", bufs=2, name=f"kg_{sg}_{dg}")
                        nc.sync.dma_start(
                            kg.rearrange("p (c o) -> p c o", o=128),
                            wk_r[:, 4 * dg : 4 * dg + 4],
                        )
                        vg = sbP.tile([128, 512], DTR, tag="vg", bufs=2, name=f"vg_{sg}_{dg}")
                        nc.sync.dma_start(
                            vg.rearrange("p (c o) -> p c o", o=128),
                            wv_r[:, 4 * dg : 4 * dg + 4],
                        )
                        for dc in range(4):
                            d = 4 * dg + dc
                            st = d == 0
                            sp = d == DCH - 1
                            rhs = xg[:, dc * SW : (dc + 1) * SW]
                            for h in range(HPC):
                                nc.tensor.matmul(
                                    q_ps[h],
                                    wq_sb[:, d * OPC + h * HD : d * OPC + (h + 1) * HD],
                                    rhs,
                                    start=st,
                                    stop=sp,
                                )
                            nc.tensor.matmul(
                                k_ps, kg[:, dc * 128 : (dc + 1) * 128], rhs,
                                start=st, stop=sp,
                            )
                            nc.tensor.matmul(
                                v_ps, vg[:, dc * 128 : (dc + 1) * 128], rhs,
                                start=st, stop=sp,
                            )

                    # RoPE: out[0:64] = r*cos - i*sin ; out[64:128] = r*sin + i*cos
                    for h in range(HPC):
                        qsb = qpool.tile([128, SW], DTR, name=f"qsb_{sg}_{h}")
                        q_tiles[(sg, h)] = qsb
                        t1 = sbP.tile([128, SW], DT, tag="rt1", bufs=2, name=f"rt1_{sg}_{h}")
                        t2 = sbP.tile([128, SW], DT, tag="rt2", bufs=2, name=f"rt2_{sg}_{h}")
                        nc.vector.tensor_mul(t1, q_ps[h], cos2_sb[:, scol])
                        nc.vector.tensor_mul(t2[0:64], q_ps[h][64:128], sinpm_sb[0:64, scol])
                        nc.vector.tensor_mul(t2[64:128], q_ps[h][0:64], sinpm_sb[64:128, scol])
                        nc.vector.tensor_add(qsb, t1, t2)
                    t1k = sbP.tile([128, SW], DT, tag="rt1", bufs=2, name=f"rt1k_{sg}")
                    t2k = sbP.tile([128, SW], DT, tag="rt2", bufs=2, name=f"rt2k_{sg}")
                    nc.vector.tensor_mul(t1k, k_ps, cos2_sb[:, scol])
                    nc.vector.tensor_mul(t2k[0:64], k_ps[64:128], sinpm_sb[0:64, scol])
                    nc.vector.tensor_mul(t2k[64:128], k_ps[0:64], sinpm_sb[64:128, scol])
                    nc.vector.tensor_add(KT_sb[:, scol], t1k, t2k)

                    # V: evacuate then transpose [hd, kv] -> [kv, hd] blocks
                    vtmp = sbP.tile([128, SW], DT, tag="vtmp", bufs=2, name=f"vtmp_{sg}")
                    nc.scalar.copy(vtmp, v_ps)
                    for jj in range(4):
                        j = 4 * sg + jj
                        tr_ps = psP.tile([128, 128], DT, tag="tr", bufs=2, name=f"trp_{j}")
                        nc.tensor.transpose(tr_ps, vtmp[:, jj * 128 : (jj + 1) * 128], ident)
                        nc.vector.tensor_copy(V_sb[:, j * 128 : (j + 1) * 128], tr_ps)

            # ------- Phases A+W interleaved: attention, then output proj + RS
            # per seq group. PSUM: A uses 6 banks (s2/attn2/den2), W uses 2.
            with (
                tc.tile_pool(name="psA", bufs=1, space="PSUM") as psA,
                tc.tile_pool(name="sbA", bufs=1) as sbA,
                tc.tile_pool(name="psW", bufs=1, space="PSUM") as psW,
                tc.tile_pool(name="sbW", bufs=1) as sbW,
            ):
                wo_sb = sbW.tile([128, 32 * OPC], DTR)  # resident woT (64KB/part)
                wo_v = wo_sb.rearrange("p (g d) -> p g d", d=OPC)
                wo_r = woT.rearrange("g p d -> p g d")
                for i in range(8):
                    nc.sync.dma_start(wo_v[:, 4 * i : 4 * i + 4], wo_r[:, 4 * i : 4 * i + 4])

                for qt in range(NSG):
                    with nc.named_scope(f"phaseA{qt}"):
                        nb = 4 * qt + 4
                        for h in range(HPC):
                            attn_ps = psA.tile([128, SW], DT, tag="attn", bufs=2, name=f"aps_{qt}_{h}")
                            den_ps = psA.tile([1, SW], DT, tag="den", bufs=2, name=f"dps_{qt}_{h}")
                            for j in range(nb):
                                s_ps = psA.tile([128, SW], DT, tag="s", bufs=2, name=f"sps_{qt}_{h}_{j}")
                                nc.tensor.matmul(
                                    s_ps,
                                    KT_sb[:, j * 128 : (j + 1) * 128],
                                    q_tiles[(qt, h)],
                                    start=True,
                                    stop=True,
                                )
                                exp_sb = sbA.tile([128, SW], DTR, tag="exp", bufs=3, name=f"exp_{qt}_{h}_{j}")
                                kk = j - 4 * qt
                                off = 128 * kk if kk > 0 else 0
                                nc.scalar.activation(
                                    exp_sb[:, off:], s_ps[:, off:], FP.Exp, scale=SCALE
                                )
                                if kk >= 0:  # diagonal block: zero kv > q triangle
                                    nc.vector.tensor_mul(
                                        exp_sb[:, off : off + 128],
                                        exp_sb[:, off : off + 128],
                                        tri_keep,
                                    )
                                nc.tensor.matmul(
                                    attn_ps[:, off:],
                                    V_sb[:, j * 128 : (j + 1) * 128],
                                    exp_sb[:, off:],
                                    start=(j == 0),
                                    stop=(j == nb - 1),
                                )
                                nc.tensor.matmul(
                                    den_ps[:, off:],
                                    ones_col,
                                    exp_sb[:, off:],
                                    start=(j == 0),
                                    stop=(j == nb - 1),
                                )
                            den_sb = sbA.tile([1, SW], DT, tag="densb", bufs=2, name=f"den_{qt}_{h}")
                            nc.scalar.copy(den_sb, den_ps)
                            rd_sb = sbA.tile([1, SW], DT, tag="rd", bufs=2, name=f"rd_{qt}_{h}")
                            nc.vector.reciprocal(rd_sb, den_sb)
                            rd_bc = sbA.tile([128, SW], DT, tag="rdbc", bufs=2, name=f"rdbc_{qt}_{h}")
                            nc.gpsimd.partition_broadcast(rd_bc, rd_sb)
                            nc.vector.tensor_mul(
                                attnT[h][:, qt * SW : (qt + 1) * SW], attn_ps, rd_bc
                            )

                    # ---- W pass for this seq group: yT[:, st cols] + RS ----
                    st = qt
                    with nc.named_scope(f"phaseW{st}"):
                        for g in range(8):
                            ysb = sbW.tile([128, 2048], DT, tag="ysb", bufs=2, name=f"ysb_{st}_{g}")
                            for dt in range(4):
                                dti = 4 * g + dt
                                yp = psW.tile([128, SW], DT, tag="yp", bufs=2, name=f"yp_{st}_{dti}")
                                for oc in range(HPC):
                                    nc.tensor.matmul(
                                        yp,
                                        wo_sb[:, dti * OPC + oc * 128 : dti * OPC + (oc + 1) * 128],
                                        attnT[oc][:, st * SW : (st + 1) * SW],
                                        start=(oc == 0),
                                        stop=(oc == HPC - 1),
                                    )
                                if dt % 2 == 0:
                                    nc.scalar.copy(ysb[:, dt * SW : (dt + 1) * SW], yp)
                                else:
                                    nc.vector.tensor_copy(ysb[:, dt * SW : (dt + 1) * SW], yp)
                            nc.sync.dma_start(
                                yT_st[st].rearrange("(g p) s -> p g s", p=128)[:, 4 * g : 4 * g + 4],
                                ysb.rearrange("p (c s) -> p c s", s=SW),
                            )
                        if st < 3:
                            nc.gpsimd.collective_compute(
                                "ReduceScatter",
                                mybir.AluOpType.add,
                                replica_groups=[list(range(N_CORES))],
                                ins=[yT_st[st][:]],
                                outs=[rs_outs[st][:]],
                            )
                        else:
                            # last seq group: scatter in quarters so earlier
                            # quarters' collectives overlap remaining matmuls
                            for qq in range(4):
                                nc.gpsimd.collective_compute(
                                    "ReduceScatter",
                                    mybir.AluOpType.add,
                                    replica_groups=[list(range(N_CORES))],
                                    ins=[yT_st[st][qq * 1024 : (qq + 1) * 1024]],
                                    outs=[rs_outs[st][qq * 128 : (qq + 1) * 128]],
                                )

                # output DMAs last: they wait on collective completion, so
                # keeping them at the end leaves the gpsimd queue free for the
                # RS triggers and broadcasts that compute depends on
                for st in range(NSG):
                    nc.gpsimd.dma_start(y_out[st], rs_outs[st][:])

    nc.compile()
    return nc


def _get_program():
    global _cached
    if _cached is None:
        _cached = build_program()
    return _cached


_ROPE_PERM = np.concatenate([np.arange(0, HD, 2), np.arange(1, HD, 2)])


def kernel(**inputs):
    x = np.asarray(inputs["x"], np.float32)
    wq = np.asarray(inputs["wq"], np.float32)
    wk = np.asarray(inputs["wk"], np.float32)
    wv = np.asarray(inputs["wv"], np.float32)
    wo = np.asarray(inputs["wo"], np.float32)
    fc = np.asarray(inputs["freqs_cos"], np.float32)
    fs = np.asarray(inputs["freqs_sin"], np.float32)

    xT = np.ascontiguousarray(x.reshape(S, DIM).T)          # [DIM, S]
    cosT = np.ascontiguousarray(fc.T)                        # [64, S]
    sinT = np.ascontiguousarray(fs.T)
    cos2 = np.concatenate([cosT, cosT], axis=0)              # [128, S]
    sinpm = np.concatenate([-sinT, sinT], axis=0)

    in_maps = []
    for c in range(N_CORES):
        wq_c = wq[c * OPC : (c + 1) * OPC].reshape(HPC, HD, DIM)[:, _ROPE_PERM]
        wqT_c = np.ascontiguousarray(wq_c.reshape(OPC, DIM).T)
        wkT_c = np.ascontiguousarray(wk[c * HD : (c + 1) * HD][_ROPE_PERM].T)
        wvT_c = np.ascontiguousarray(wv[c * HD : (c + 1) * HD].T)
        wo_c = wo[:, c * OPC : (c + 1) * OPC]                # [DIM, 512]
        woT_blk = np.ascontiguousarray(
            wo_c.reshape(32, 128, HPC, 128).transpose(0, 3, 2, 1)
        ).reshape(32, 128, OPC)
        in_maps.append(
            dict(
                xT=xT, wqT=wqT_c, wkT=wkT_c, wvT=wvT_c, woT=woT_blk,
                cos2=cos2, sinpm=sinpm,
            )
        )

    nc = _get_program()
    res = run_bass_kernel_spmd(nc, in_maps, list(range(N_CORES)))
    global last_results
    last_results = res

    yT = np.empty((DIM, S), np.float32)
    for c in range(N_CORES):
        shard = res.results[c]["y_shard"]                    # [4, 512, 512]
        for st in range(3):
            yT[512 * c : 512 * (c + 1), 512 * st : 512 * (st + 1)] = shard[st]
        # st=3 was reduce-scattered in four 1024-row quarters
        for qq in range(4):
            yT[1024 * qq + 128 * c : 1024 * qq + 128 * (c + 1), 1536:2048] = (
                shard[3][128 * qq : 128 * (qq + 1)]
            )
    return np.ascontiguousarray(yT.T).reshape(B, S, DIM)
